# revision 20
# baseline (speedup 1.0000x reference)
"""TRN2 Bass/Tile kernel for nn_DotProductAttention (softmax over the QUERY axis).

reference:
    scores  = einsum('bqd,bkd->bqk', q, k) / sqrt(64)
    weights = softmax(scores, axis=1)          # over q, NOT k!
    out     = einsum('bqk,bkd->bqd', weights, v)

Because the softmax normalizes over q for each (b, k) column, we work with the
transposed score matrix T = K @ Q^T (shape [k, q]): the reduction axis (q) is
then the free axis, which the ACT accum_out reduction handles for free, and the
normalizer Z[k] lives on the contraction axis of the second matmul so it can be
folded into V (V' = V / Z) instead of rescaling the whole [k, q] tile.

Sharding: B=16 batches, data-parallel over 8 cores => 2 batches per core.
The two batches of a core are packed into the two 64-partition halves of
[128, *] tiles ((b, d) packing), which lets pairs of matmuls run concurrently
in disjoint PE-array row strips (scores) / column strips (AV).
"""

import math
from contextlib import ExitStack

import numpy as np

import concourse.bass as bass  # noqa: F401  (kept for symmetry with docs)
import concourse.mybir as mybir
import concourse.tile as tile
from bass_rust import add_dep_helper
from concourse import bacc, bass_utils
from concourse.masks import make_identity

FP32 = mybir.dt.float32
BF16 = mybir.dt.bfloat16

N_CORES = 8
B_FULL = 16
BPC = B_FULL // N_CORES  # batches per core = 2
S = 2048
D = 64
NCH = S // 128  # 16 key chunks of 128
SCALE = 1.0 / math.sqrt(D)


def emit_kernel(ctx: ExitStack, tc, q, k, v, o):
    """Emit the per-core Tile program. q/k/v/o are DRAM APs of [BPC, S, D] f32."""
    nc = tc.nc

    const_pool = ctx.enter_context(tc.tile_pool(name="const", bufs=1))
    big = ctx.enter_context(tc.tile_pool(name="big", bufs=1))
    # PSUM: phase B1 double-buffers two [128,2048] score tiles (all 8 banks);
    # phase B2 reuses the same pool for the [128,2048] O^T accumulator and
    # the [128,128] transpose tiles.
    ps = ctx.enter_context(tc.tile_pool(name="ps", bufs=2, space="PSUM"))

    ident = const_pool.tile([128, 128], FP32, name="ident")
    make_identity(nc, ident)
    zw = const_pool.tile([128, 128], BF16, name="zw")
    nc.vector.memset(zw[:], 0.0)

    # (b,d)-packed transposed operands: partitions 0:64 = batch0 d, 64:128 = batch1 d.
    QT = big.tile([128, S], BF16, name="QT")
    KT = big.tile([128, S], BF16, name="KT")
    # staging for Q/K chunks in (m, b, d) column layout, s on partitions
    qstage = big.tile([128, S], FP32, name="qstage")
    kstage = big.tile([128, S], FP32, name="kstage")
    # V chunks [128 k, 64 d] (f32 as loaded) and Vs = V / Z (bf16)
    V = big.tile([128, BPC * NCH * D], FP32, name="V")
    Vs = big.tile([128, BPC * NCH * D], BF16, name="Vs")
    # per (b, chunk) stats columns: [z, 1/z]
    stats = big.tile([128, BPC * NCH * 2], FP32, name="stats")
    # E[(b*NCH+i)*S :+ S] = exp(scores/sqrt(D)): [128 k, 2048 q] bf16, fully resident
    E = big.tile([128, BPC * NCH * S], BF16, name="E")
    # O^T staging ((b,d) packed on partitions, q on free), f32
    OT = big.tile([128, S], FP32, name="OT")
    # O in natural layout: column chunk m holds [q-tile m, (b d)]
    O_all = big.tile([128, S], FP32, name="O_all")

    # bf16 copies of the staged (s, (b d)) matrices; transposed per-chunk via
    # SBUF->SBUF xbar DMAs
    qbf = big.tile([128, S], BF16, name="qbf")
    kbf = big.tile([128, S], BF16, name="kbf")

    # ---------------- phase A: load + transpose Q/K, load V ----------------
    # The two HWDGE sequencers (sync, scalar) are the serial resource at the
    # head, and the scalar queue also carries B1's ACTIVATEs — so the scalar
    # queue gets only the cheap/early work and V rides SWDGE on the idle
    # GPSIMD queue. Per-(half, b) stage DMAs; Q/K transposed chunk-by-chunk
    # with xbar DMAs, most-urgent first (scores chunk 0 reads ALL of QT + KT
    # chunk 0).
    MPQ = NCH // 2  # chunks per half
    for Q in range(2):
        ssl = slice(Q * MPQ * 128, (Q + 1) * MPQ * 128)
        csl = slice(Q * MPQ * 128, (Q + 1) * MPQ * 128)
        for b in range(BPC):
            eng = nc.sync if b == 0 else nc.scalar
            for src, stg in ((q, qstage), (k, kstage)):
                eng.dma_start(
                    stg[:, csl].rearrange("p (m b d) -> p m b d", m=MPQ, b=BPC, d=D)[
                        :, :, b, :
                    ],
                    src[b, ssl, :].rearrange("(m p) d -> p m d", p=128),
                )
    for b in range(BPC):
        nc.gpsimd.dma_start(
            V[:].rearrange("p (b m d) -> p b m d", b=BPC, m=NCH)[:, b, :, :],
            v[b].rearrange("(m p) d -> p m d", p=128),
        )
    for stg, bft in ((qstage, qbf), (kstage, kbf)):
        for h in range(2):
            nc.vector.tensor_copy(
                bft[:, h * 1024 : (h + 1) * 1024], stg[:, h * 1024 : (h + 1) * 1024]
            )

    def xbar(bft, dst, m):
        return (
            dst[:, m * 128 : (m + 1) * 128],
            bft[:, m * 128 : (m + 1) * 128],
        )

    sync_list = (
        [("q", m) for m in range(4)]
        + [("k", 0)]
        + [("q", m) for m in range(8, 12)]
        + [("k", m) for m in range(1, 8)]
        + [("k", m) for m in range(8, 16)]
    )
    scal_list = [("q", m) for m in range(4, 8)] + [("q", m) for m in range(12, 16)]
    srcmap = {"q": (qbf, QT), "k": (kbf, KT)}
    for eng, lst in ((nc.sync, sync_list), (nc.scalar, scal_list)):
        for t, m in lst:
            bft, dst = srcmap[t]
            out_ap, in_ap = xbar(bft, dst, m)
            eng.dma_start_transpose(out=out_ap, in_=in_ap)

    # ---------------- phase B1: scores -> exp, double-buffered --------------
    # Two [128,2048] score tiles rotate through all 8 PSUM banks; one N=2048
    # exp per (batch, chunk) with accum_out giving the softmax normalizer Z
    # directly (softmax axis == free axis).
    for i in range(NCH):
        for b in range(BPC):
            sct = ps.tile([128, S], FP32, tag="ps", name=f"sc{i}_{b}")
            for j in range(4):
                nc.tensor.matmul(
                    sct[:, j * 512 : (j + 1) * 512],
                    lhsT=KT[b * 64 : (b + 1) * 64, i * 128 : (i + 1) * 128],
                    rhs=QT[b * 64 : (b + 1) * 64, j * 512 : (j + 1) * 512],
                    start=True,
                    stop=True,
                )
            sb = (b * NCH + i) * 2
            eb = (b * NCH + i) * S
            nc.scalar.activation(
                E[:, eb : eb + S],
                sct[:],
                mybir.ActivationFunctionType.Exp,
                scale=SCALE,
                accum_out=stats[:, sb : sb + 1],
            )
            vb = (b * NCH + i) * D
            nc.vector.reciprocal(stats[:, sb + 1 : sb + 2], stats[:, sb : sb + 1])
            nc.vector.tensor_scalar_mul(
                Vs[:, vb : vb + D], V[:, vb : vb + D], stats[:, sb + 1 : sb + 2]
            )

    # ---------------- phase B2: dense AV accumulation -----------------------
    pot = ps.tile([128, S], FP32, tag="ps", name="pot")
    # Open each accumulator bank with a full-128-partition zeroing matmul
    # (zero weights): writes 0 everywhere and sets has_written for the whole
    # bank on every execution, so the partition-sliced AV matmuls below can
    # all accumulate with start=False regardless of how the HW scopes the
    # first_mm bank-clear across partitions.
    zmm = []
    for j in range(4):
        zmm.append(
            nc.tensor.matmul(
                pot[:, j * 512 : (j + 1) * 512],
                lhsT=zw[:],
                rhs=QT[:, 0:512],
                start=True,
                stop=False,
                skip_group_check=True,
            )
        )
    for i in range(NCH):
        for j in range(4):
            for b in range(BPC):
                # O^T[(b,d), q] += Vs_i^T @ E_i ; b0 -> PE cols 0:63,
                # b1 -> cols 64:127 (concurrent via col tiling)
                vb = (b * NCH + i) * D
                eb = (b * NCH + i) * S
                mm = nc.tensor.matmul(
                    pot[b * 64 : (b + 1) * 64, j * 512 : (j + 1) * 512],
                    lhsT=Vs[:, vb : vb + D],
                    rhs=E[:, eb + j * 512 : eb + (j + 1) * 512],
                    start=False,
                    stop=(i == NCH - 1 and b == BPC - 1),
                    skip_group_check=True,
                )
                if i == 0:
                    add_dep_helper(
                        mm.ins,
                        zmm[j].ins,
                        sync=False,
                        reason="AV accumulation after bank-opening zero matmul",
                    )

    # ---------------- phase C: unpack O^T -> O, store ----------------------
    for j in range(4):
        nc.vector.tensor_copy(
            OT[:, j * 512 : (j + 1) * 512], pot[:, j * 512 : (j + 1) * 512]
        )
    o_view = O_all[:].rearrange("p (m b d) -> p m b d", m=NCH, b=BPC, d=D)
    for m in range(NCH):
        ptc = ps.tile([128, 128], FP32, tag="ps", name=f"ptc_{m}")
        nc.tensor.transpose(ptc[:], OT[:, m * 128 : (m + 1) * 128], ident[:])
        nc.vector.tensor_copy(O_all[:, m * 128 : (m + 1) * 128], ptc[:])
        if m % 4 == 3:
            # stream the finished quarter out while later chunks transpose
            for b in range(BPC):
                nc.sync.dma_start(
                    o[b, (m - 3) * 128 : (m + 1) * 128, :].rearrange(
                        "(m p) d -> p m d", p=128
                    ),
                    o_view[:, m - 3 : m + 1, b, :],
                )


_CACHE: dict = {}


def build_program():
    if "nc" in _CACHE:
        return _CACHE["nc"]
    nc = bacc.Bacc("TRN2", target_bir_lowering=False, debug=False)
    q = nc.dram_tensor("q", [BPC, S, D], FP32, kind="ExternalInput").ap()
    k = nc.dram_tensor("k", [BPC, S, D], FP32, kind="ExternalInput").ap()
    v = nc.dram_tensor("v", [BPC, S, D], FP32, kind="ExternalInput").ap()
    o = nc.dram_tensor("o", [BPC, S, D], FP32, kind="ExternalOutput").ap()
    with tile.TileContext(nc) as tc:
        with ExitStack() as ctx:
            emit_kernel(ctx, tc, q, k, v, o)
    nc.compile()
    _CACHE["nc"] = nc
    return nc


def make_in_maps(q, k, v):
    q = np.ascontiguousarray(q, dtype=np.float32)
    k = np.ascontiguousarray(k, dtype=np.float32)
    v = np.ascontiguousarray(v, dtype=np.float32)
    assert q.shape == (B_FULL, S, D), q.shape
    return [
        {
            "q": np.ascontiguousarray(q[c * BPC : (c + 1) * BPC]),
            "k": np.ascontiguousarray(k[c * BPC : (c + 1) * BPC]),
            "v": np.ascontiguousarray(v[c * BPC : (c + 1) * BPC]),
        }
        for c in range(N_CORES)
    ]


def kernel(q, k, v, _trace=False):
    nc = build_program()
    in_maps = make_in_maps(q, k, v)
    res = bass_utils.run_bass_kernel_spmd(
        nc, in_maps, core_ids=list(range(N_CORES)), trace=_trace
    )
    out = np.concatenate([r["o"] for r in res.results], axis=0)
    if _trace:
        return out, res
    return out
